# revision 1
# baseline (speedup 1.0000x reference)
import numpy as np
import jax
import jax.numpy as jnp
from functools import partial

# nn_CGIterator: 4 chained CG iterations over node-local features.
# Sharding: node axis (axis 0 of f0/f1/f2) split 8 ways across NeuronCores;
# U, gamma, W_in, W_out replicated. All ops are node-local -> no collectives.

N = 50000
K = 128
I = 4
EPS = 1e-6
NCORES = 8
NPAD = 50176  # 8 * 6272


def _cg_shard(f0, f1, f2, U, gamma, W_in, W_out):
    feats = [f0, f1, f2]
    for i in range(I):
        res = feats
        h = [x * jax.lax.rsqrt(jnp.mean(x * x, axis=1, keepdims=True) + EPS)
             * gamma[i, l] for l, x in enumerate(feats)]
        h = [jnp.einsum('nmk,kp->nmp', h[l], W_in[i, l]) for l in range(3)]
        hc = jnp.concatenate(h, axis=1)  # [n, 9, 2K]
        tp = jnp.einsum('abc,nak,nbk->nck', U, hc, hc)
        parts = jnp.split(tp, [1, 4], axis=1)
        feats = [r + jnp.einsum('nmp,pk->nmk', parts[l], W_out[i, l])
                 for l, r in enumerate(res)]
    return jnp.concatenate(feats, axis=1)  # [n, 9, K]


_pmapped = jax.pmap(_cg_shard, axis_name='x',
                    in_axes=(0, 0, 0, None, None, None, None))


def kernel(f0, f1, f2, U, gamma, W_in, W_out):
    f0 = np.asarray(f0, np.float32)
    f1 = np.asarray(f1, np.float32)
    f2 = np.asarray(f2, np.float32)
    n = f0.shape[0]
    npc = NPAD // NCORES

    def shard(x):
        xp = np.zeros((NPAD,) + x.shape[1:], np.float32)
        xp[:n] = x
        return xp.reshape(NCORES, npc, *x.shape[1:])

    out = _pmapped(shard(f0), shard(f1), shard(f2),
                   jnp.asarray(U), jnp.asarray(gamma),
                   jnp.asarray(W_in), jnp.asarray(W_out))
    out = np.asarray(out).reshape(NPAD, 9, K)[:n]
    return out

